# revision 21
# baseline (speedup 1.0000x reference)
"""Multi-head causal attention (B=2, T=2048, E=1024, H=16, D=64) on 8 TRN2 cores.

Sharding: tensor-parallel over heads. Core c owns heads {2c, 2c+1} for both
batches. Each core computes its heads' q/k/v projections, causal attention,
and a partial output projection z_c = out_c @ Wo[:, 128c:128c+128].T.
Host combines: z = sum_c z_c + bo.

Note the reference computes wei = K @ Q^T, i.e. output token t attends over
s <= t with logits k_t . q_s. We compute ST[s, t] = q_s . k_t (s on
partitions) so that the A@V matmul needs no transposes, and get the softmax
denominator via a ones-column appended to V.
"""

import numpy as np
import ml_dtypes

import concourse.bacc as bacc
import concourse.mybir as mybir
import concourse.tile as tile
from concourse.bass_utils import run_bass_kernel_spmd
from concourse.masks import make_identity


def _make_runner(nc):
    """Persistent jitted SPMD callable (avoids per-call jit re-trace)."""
    import jax
    from jax.sharding import Mesh, NamedSharding, PartitionSpec
    try:
        from jax.experimental.shard_map import shard_map
    except ImportError:
        shard_map = jax.shard_map
    from concourse.bass2jax import (_bass_exec_p, install_neuronx_cc_hook,
                                    partition_id_tensor)

    install_neuronx_cc_hook()
    partition_name = (nc.partition_id_tensor.name
                      if nc.partition_id_tensor else None)
    in_names, out_names, out_avals, zero_outs = [], [], [], []
    for alloc in nc.m.functions[0].allocations:
        if not isinstance(alloc, mybir.MemoryLocationSet):
            continue
        name = alloc.memorylocations[0].name
        if alloc.kind == "ExternalInput":
            if name != partition_name:
                in_names.append(name)
        elif alloc.kind == "ExternalOutput":
            shape = tuple(alloc.tensor_shape)
            dtype = mybir.dt.np(alloc.dtype)
            out_names.append(name)
            out_avals.append(jax.core.ShapedArray(shape, dtype))
            zero_outs.append(np.zeros(shape, dtype))
    n_params = len(in_names)
    all_in = list(in_names) + list(out_names)
    if partition_name is not None:
        all_in.append(partition_name)

    def _body(*args):
        operands = list(args)
        if partition_name is not None:
            operands.append(partition_id_tensor())
        return tuple(_bass_exec_p.bind(
            *operands, out_avals=tuple(out_avals), in_names=tuple(all_in),
            out_names=tuple(out_names), lowering_input_output_aliases=(),
            sim_require_finite=True, sim_require_nnan=True, nc=nc))

    devices = jax.devices()[:N_CORES]
    mesh = Mesh(np.asarray(devices), ("core",))
    spec = NamedSharding(mesh, PartitionSpec("core"))
    rspec = NamedSharding(mesh, PartitionSpec())
    # inputs identical on every core are sent once and replicated
    replicated = {"xt", "mask"}
    in_specs = tuple(
        (PartitionSpec() if nm in replicated else PartitionSpec("core"))
        for nm in in_names) + (PartitionSpec("core"),) * len(out_names)
    fn = jax.jit(
        shard_map(_body, mesh=mesh, in_specs=in_specs,
                  out_specs=(PartitionSpec("core"),) * len(out_names),
                  check_rep=False),
        keep_unused=True)
    zeros_dev = [
        jax.device_put(np.zeros((N_CORES * z.shape[0], *z.shape[1:]), z.dtype),
                       spec) for z in zero_outs
    ]

    def run(in_maps):
        concat = [
            jax.device_put(np.asarray(in_maps[0][nm]), rspec)
            if nm in replicated else
            jax.device_put(
                np.concatenate([np.asarray(in_maps[c][nm])
                                for c in range(N_CORES)], axis=0), spec)
            for nm in in_names
        ]
        outs = fn(*concat, *zeros_dev)
        fulls = [np.asarray(outs[i]).reshape(N_CORES, *out_avals[i].shape)
                 for i in range(len(out_names))]
        return [{nm: fulls[i][c] for i, nm in enumerate(out_names)}
                for c in range(N_CORES)]

    return run

N_CORES = 8
B, T, E = 2, 2048, 1024
H, D = 16, 64
HPC = H // N_CORES          # heads per core = 2
F = HPC * D                 # local feature cols = 128
TBLK = 512                  # t-block width for stage A
NTB = T // TBLK             # 4
NSC = T // 128              # s-chunks = 16
NEC = E // 128              # e-chunks = 8
EXP_BIAS = -2.0             # exp(S + EXP_BIAS); cancels in softmax, guards overflow

F32 = mybir.dt.float32
F16 = mybir.dt.float16
F32R = mybir.dt.float32r
BF16 = mybir.dt.bfloat16
EXP = mybir.ActivationFunctionType.Exp


def build_nc(rep=1, cfg=None):
    cfg = dict(cfg or {})
    any_copy = cfg.get("any_copy", False)
    evict = cfg.get("evict", "mixed")  # mixed|zscalar|zvector
    sp_bufs = cfg.get("sp_bufs", 2)
    op_bufs = cfg.get("op_bufs", 2)
    misc_bufs = cfg.get("misc_bufs", None)  # if set, tp+zp merged [128,512] x misc_bufs
    pt_bufs = cfg.get("pt_bufs", 4)
    out_q = cfg.get("out_q", "scalar")  # engine for output DMAs
    xt_bf16 = cfg.get("xt_bf16", False)
    skip_z = cfg.get("skip_z", False)
    skip_b = cfg.get("skip_b", False)
    skip_attn = cfg.get("skip_attn", False)
    td_form = cfg.get("td_form", False)
    sp_wide = cfg.get("sp_wide", False)
    qk_bf16 = cfg.get("qk_bf16", False)   # bf16 qT2/kT2 + r=3 S trim
    v_bf16 = cfg.get("v_bf16", False)     # bf16 vT2 (faster PE transpose)
    exp_split = cfg.get("exp_split", False)  # exp only live cols of diag pair
    xtp_bufs = cfg.get("xtp_bufs", 36)
    xt_coal = cfg.get("xt_coal", False)  # one DMA per T-quarter (all E chunks)
    interleave = cfg.get("interleave", False)  # proj round 0 -> tb0/1 -> ...
    defer_z = cfg.get("defer_z", False)  # z-stage emitted inside next tb
    nc = bacc.Bacc("TRN2", target_bir_lowering=False, debug=False,
                   num_devices=N_CORES)

    xt = nc.dram_tensor("xt", [B, E, T], BF16 if xt_bf16 else F32R,
                        kind="ExternalInput").ap()
    # host pre-swizzles weights to [128, NEC*F]: w[p, e*F+f] = W[e*128+p, f]
    wdt = BF16 if xt_bf16 else F32R
    wq = nc.dram_tensor("wq", [128, NEC * F], wdt, kind="ExternalInput").ap()
    wk = nc.dram_tensor("wk", [128, NEC * F], wdt, kind="ExternalInput").ap()
    wv = nc.dram_tensor("wv", [128, NEC * F], wdt, kind="ExternalInput").ap()
    wot = nc.dram_tensor("wot", [F, E], F32R, kind="ExternalInput").ap()
    mask = nc.dram_tensor("mask", [128, 128], BF16, kind="ExternalInput").ap()
    zp = nc.dram_tensor("zp", [B, T, E], F16, kind="ExternalOutput").ap()

    with tile.TileContext(nc) as tc:
        with (
            tc.tile_pool(name="const", bufs=1) as cpool,
            tc.tile_pool(name="xtp", bufs=xtp_bufs) as xtp,
            tc.tile_pool(name="proj", bufs=2) as projp,
            tc.tile_pool(name="v2p", bufs=2 * NSC) as v2p,
            tc.tile_pool(name="ptp", bufs=pt_bufs) as ptp,
            tc.tile_pool(name="smallp", bufs=4) as smallp,
            tc.tile_pool(name="zsbp", bufs=3) as zsbp,
            tc.tile_pool(name="ps_s", bufs=sp_bufs, space="PSUM") as ps_s,
            tc.tile_pool(name="ps_o", bufs=op_bufs, space="PSUM") as ps_o,
            tc.tile_pool(name="ps_t", bufs=(misc_bufs or 2), space="PSUM") as ps_t,
        ):
            # ---- constants (loaded once) ----
            ident = cpool.tile([128, 128], F32, tag="ident")
            make_identity(nc, ident[:])
            if v_bf16:
                identb = cpool.tile([128, 128], BF16, tag="identb")
                make_identity(nc, identb[:])
            else:
                identb = ident
            ebias = cpool.tile([128, 1], F32, tag="ebias")
            nc.vector.memset(ebias[:], EXP_BIAS)
            # host pre-swizzled walls: straight [128, NEC*F] DMAs (2KB lines)
            wq_sb = []
            wk_sb = []
            wv_sb = []
            for lst, wsrc, nm in ((wq_sb, wq, "wq"), (wk_sb, wk, "wk"),
                                  (wv_sb, wv, "wv")):
                t_ = cpool.tile([128, NEC * F], BF16 if xt_bf16 else F32R,
                                tag=f"wall{nm}")
                nc.sync.dma_start(t_[:], wsrc)
                for e in range(NEC):
                    lst.append(t_[:, e * F:(e + 1) * F])
            wot_sb = cpool.tile([F, E], F32R, tag="wot")
            nc.sync.dma_start(wot_sb[:], wot)
            mask_sb = cpool.tile([128, 128], BF16, tag="mask")
            nc.sync.dma_start(mask_sb[:], mask)

            def body():
                pending = []

                def flush_pending():
                    while pending:
                        pending.pop(0)()

                xthb = {}
                if xt_coal:
                    # one coalesced DMA per T-quarter covering all 8 E-chunks
                    # (HWDGE setup is ~600ns serialized per dma_start); both
                    # batches issued up-front so batch 1's load overlaps
                    # batch 0's compute
                    for b in range(B):
                        xth = [[None] * 4 for _ in range(NEC)]
                        for qt in range(4):
                            t_ = xtp.tile([128, NEC * (T // 4)],
                                          BF16 if xt_bf16 else F32R,
                                          tag="xtq", bufs=8 if xt_bf16 else 5)
                            nc.sync.dma_start(
                                t_.rearrange("p (a c) -> p a c", a=NEC),
                                xt[b, :, qt * (T // 4):(qt + 1) * (T // 4)]
                                .rearrange("(a p) c -> p a c", p=128))
                            for e in range(NEC):
                                xth[e][qt] = t_[:, e * (T // 4):
                                                (e + 1) * (T // 4)]
                        xthb[b] = xth
                for b in range(B):
                    # ---- load transposed activations (t-halves so the first
                    # projection group can start after half the input DMA) ----
                    if xt_coal:
                        xth = xthb[b]
                    else:
                        xth = [[None] * 4 for _ in range(NEC)]
                        for qt in range(4):
                            for e in range(NEC):
                                t_ = xtp.tile([128, T // 4],
                                              BF16 if xt_bf16 else F32R,
                                              tag="xt")
                                nc.sync.dma_start(
                                    t_[:], xt[b, e * 128:(e + 1) * 128,
                                              qt * (T // 4):(qt + 1) * (T // 4)])
                                xth[e][qt] = t_

                    # ---- projections: qT2/kT2/vT2 [128(f), T] ----
                    heads = {}
                    for nm in ("q", "k", "v"):
                        if nm == "v":
                            pdt = BF16 if v_bf16 else F32
                        else:
                            pdt = BF16 if qk_bf16 else F32R
                        heads[nm] = projp.tile([128, T], pdt, tag=f"{nm}T2",
                                               name=f"{nm}T2")
                    qT2, kT2, vT2 = heads["q"], heads["k"], heads["v"]
                    v2 = [None] * NSC

                    def proj_round(tp2):
                        for nm, wsb in (("q", wq_sb), ("k", wk_sb),
                                        ("v", wv_sb)):
                            dst = heads[nm]
                            ps = ps_s.tile([128, 1024], F32, tag="sp",
                                           name="ps")
                            for half in range(2):
                                c0 = tp2 * 1024 + half * 512
                                for e in range(NEC):
                                    nc.tensor.matmul(
                                        ps[:, half * 512:(half + 1) * 512],
                                        wsb[e],
                                        xth[e][c0 // 512][:],
                                        start=(e == 0), stop=(e == NEC - 1))
                            (nc.any if any_copy else nc.vector).tensor_copy(
                                dst[:, tp2 * 1024:(tp2 + 1) * 1024], ps[:])

                    # ---- v2[s]: [128(s), 130] bf16 = [1|v_h0|1|v_h1] ----
                    def v2_round(s0, s1):
                        for s in range(s0, s1):
                            tpw = ps_t.tile([128, 512 if not v_bf16 else 1024],
                                            BF16 if v_bf16 else F32, tag="tp",
                                            name="tpw")
                            tp_ = tpw[:, 0:128]
                            nc.tensor.matmul(tp_[:],
                                             vT2[:, s * 128:(s + 1) * 128],
                                             identb[:], is_transpose=True)
                            v2t = v2p.tile([128, 130], BF16, tag="v2",
                                           name="v2t")
                            v2r = v2t.rearrange("p (g c) -> p g c", g=2)
                            nc.vector.memset(v2r[:, :, 64:65], 1.0)
                            nc.vector.tensor_copy(
                                v2r[:, :, 0:64],
                                tp_.rearrange("p (g c) -> p g c", g=2))
                            v2[s] = v2t

                    # ---- attention (stage B in outT form) ----
                    def attn_tb(tb):
                        slast = 4 * tb + 3
                        po = {}
                        for h in range(2 if not skip_b else 0):
                            if td_form:
                                po_t = ps_o.tile([128, 260], F32, tag="op")
                            else:
                                po_t = ps_o.tile([65, 512], F32, tag="op")
                            po[h] = po_t
                        npairs = 2 * tb + 2
                        for p in range(npairs):
                            pts = []
                            for h in range(2):
                                ps = ps_s.tile([128, 1024], F32, tag="sp")
                                for dp in range(2):
                                    si = 2 * p + dp
                                    r = si - 4 * tb
                                    # trim causally-dead columns where fp32r
                                    # still streams at 1 cyc/row (N >= 256);
                                    # bf16 streams 1 cyc/row at any N
                                    trim_rs = (1, 2, 3) if qk_bf16 else (1, 2)
                                    c0 = 128 * r if r in trim_rs else 0
                                    nc.tensor.matmul(
                                        ps[:, dp * 512 + c0:(dp + 1) * 512],
                                        qT2[64 * h:64 * h + 64,
                                            si * 128:(si + 1) * 128],
                                        kT2[64 * h:64 * h + 64,
                                            tb * 512 + c0:(tb + 1) * 512],
                                        start=True, stop=True)
                                pt = ptp.tile([128, 1024], BF16, tag="pt")
                                if exp_split and p == 2 * tb + 1:
                                    # diagonal (r=2, r=3) pair: exp only the
                                    # causally-live columns of each chunk
                                    for dp in range(2):
                                        rr = 2 * p + dp - 4 * tb
                                        ec0 = dp * 512 + 128 * rr
                                        nc.scalar.activation(
                                            pt[:, ec0:(dp + 1) * 512],
                                            ps[:, ec0:(dp + 1) * 512], EXP,
                                            bias=ebias[:])
                                else:
                                    nc.scalar.activation(pt[:], ps[:], EXP,
                                                         bias=ebias[:])
                                for dp in range(2):
                                    si = 2 * p + dp
                                    r = si - 4 * tb
                                    if 0 <= r < 4:
                                        sl = pt[:, dp * 512 + r * 128:
                                                dp * 512 + (r + 1) * 128]
                                        meng = (nc.gpsimd if cfg.get("mask_pool")
                                                else nc.vector)
                                        meng.tensor_mul(sl, sl, mask_sb[:])
                                pts.append(pt)
                            for dp in range(2 if not skip_b else 0):
                                si = 2 * p + dp
                                for h in range(2):
                                    if td_form:
                                        # po[h] is [128(t), 260]: j-th chunk at
                                        # cols 130*(j%2); heads share tile pair
                                        for j in range(4):
                                            tcg = 4 * tb + j
                                            if si > tcg:
                                                continue
                                            jj = j % 2
                                            dst = po[j // 2][:, jj * 130 + h * 65:
                                                             jj * 130 + (h + 1) * 65]
                                            nc.tensor.matmul(
                                                dst,
                                                pts[h][:, dp * 512 + j * 128:
                                                       dp * 512 + (j + 1) * 128],
                                                v2[si][:, h * 65:(h + 1) * 65],
                                                start=(si == 0 and h == 0),
                                                stop=(si == tcg),
                                                skip_group_check=True)
                                    else:
                                        r = si - 4 * tb
                                        c0 = max(r, 0) * 128
                                        nc.tensor.matmul(
                                            po[h][:, c0:512],
                                            v2[si][:, h * 65:(h + 1) * 65],
                                            pts[h][:, dp * 512 + c0:
                                                   (dp + 1) * 512],
                                            start=(si == 0), stop=(si == slast),
                                            skip_group_check=True)

                        # ---- normalize (rows 1:65 / row 0) + partial z ----
                        if not skip_b and not skip_z:
                            outT = smallp.tile([128, 512], F32R, tag="outT")
                            for h in range(2):
                                rrow = smallp.tile([1, 512], F32, tag="rrow")
                                nc.vector.reciprocal(rrow[:], po[h][64:65, :])
                                rbc = smallp.tile([64, 512], F32, tag="rbc")
                                nc.gpsimd.partition_broadcast(rbc[:], rrow[:])
                                nc.vector.tensor_mul(
                                    outT[64 * h:64 * h + 64, :],
                                    po[h][0:64, :], rbc[:])
                            for jp2 in range(2):
                                zsb = zsbp.tile([128, 2048], F16, tag="zsb")
                                for jj in range(2):
                                    j = 2 * jp2 + jj
                                    for eb in range(2):
                                        zps = ps_t.tile([128, 512], F32,
                                                        tag="tp")
                                        nc.tensor.matmul(
                                            zps[:],
                                            outT[:, j * 128:(j + 1) * 128],
                                            wot_sb[:, eb * 512:(eb + 1) * 512],
                                            start=True, stop=True)
                                        dstsl = zsb[:, jj * 1024 + eb * 512:
                                                    jj * 1024 + (eb + 1) * 512]
                                        if evict == "zscalar":
                                            nc.scalar.copy(dstsl, zps[:])
                                        elif evict == "zvector":
                                            nc.vector.tensor_copy(dstsl, zps[:])
                                        elif any_copy:
                                            nc.any.tensor_copy(dstsl, zps[:])
                                        elif eb == 0:
                                            nc.vector.tensor_copy(dstsl, zps[:])
                                        else:
                                            nc.scalar.copy(dstsl, zps[:])
                                t0r = (4 * tb + 2 * jp2) * 128
                                getattr(nc, out_q).dma_start(
                                    zp[b, t0r:t0r + 256, :]
                                    .rearrange("(a p) c -> p a c", p=128),
                                    zsb.rearrange("p (a c) -> p a c", a=2))

                    if interleave:
                        # attention for t-blocks 0/1 needs only the first
                        # projection round (cols 0-1023): emit it early so
                        # exp (ACT) overlaps the second projection round;
                        # end on tb=2 (6 pairs) instead of tb=3 (8) to
                        # shorten the drain tail
                        proj_round(0)
                        v2_round(0, 8)
                        if not skip_attn:
                            attn_tb(0)
                            attn_tb(1)
                        proj_round(1)
                        v2_round(8, NSC)
                        if not skip_attn:
                            attn_tb(3)
                            attn_tb(2)
                    else:
                        proj_round(0)
                        proj_round(1)
                        v2_round(0, NSC)
                        for tb in range(NTB if not skip_attn else 0):
                            attn_tb(tb)

            if rep == 1:
                body()
            else:
                with tc.For_i(0, rep, 1):
                    body()

    nc.compile()
    return nc


def make_in_maps(inputs, Wk, Wq, Wv, Wo, xt_bf16=False):
    """Shard full inputs into per-core input maps."""
    wdt = ml_dtypes.bfloat16 if xt_bf16 else np.float32
    xt = np.ascontiguousarray(inputs.transpose(0, 2, 1)).astype(wdt)
    scale = np.float32(D ** -0.5)
    tri = (np.arange(128)[None, :] >= np.arange(128)[:, None])
    mask = tri.astype(ml_dtypes.bfloat16)
    def swz(w):
        # [E, F] -> [128, NEC*F]: out[p, e*F+f] = w[e*128+p, f]
        nec = E // 128
        return np.ascontiguousarray(
            w.reshape(nec, 128, F).transpose(1, 0, 2).reshape(128, nec * F))

    in_maps = []
    for c in range(N_CORES):
        h0 = HPC * c
        wq2 = np.ascontiguousarray(
            np.concatenate([Wq[h0 + i] for i in range(HPC)], axis=1))
        wk2 = np.ascontiguousarray(
            np.concatenate([Wk[h0 + i] for i in range(HPC)], axis=1)) * scale
        wv2 = np.ascontiguousarray(
            np.concatenate([Wv[h0 + i] for i in range(HPC)], axis=1))
        wot = np.ascontiguousarray(Wo[:, F * c:F * (c + 1)].T)
        in_maps.append({
            "xt": xt,
            "wq": swz(wq2).astype(wdt),
            "wk": swz(wk2).astype(wdt),
            "wv": swz(wv2).astype(wdt),
            "wot": wot.astype(np.float32),
            "mask": mask,
        })
    return in_maps


_NC = None
_RUN = None
DEFAULT_CFG = {"any_copy": True, "out_q": "sync", "xt_bf16": False}


def kernel(inputs, Wk, Wq, Wv, Wo, bo):
    global _NC, _RUN
    if _NC is None:
        _NC = build_nc(cfg=DEFAULT_CFG)
    in_maps = make_in_maps(inputs, Wk, Wq, Wv, Wo,
                           xt_bf16=DEFAULT_CFG["xt_bf16"])
    try:
        if _RUN is None:
            _RUN = _make_runner(_NC)
        results = _RUN(in_maps)
    except Exception:
        _RUN = False if _RUN is None else _RUN
        res = run_bass_kernel_spmd(_NC, in_maps,
                                   core_ids=list(range(N_CORES)))
        results = res.results
    z = np.zeros((B, T, E), dtype=np.float32)
    for c in range(N_CORES):
        z += results[c]["zp"].astype(np.float32)
    return z + bo.astype(np.float32)



# revision 25
# speedup vs baseline: 1.2124x; 1.2124x over previous
"""Multi-head causal attention (B=2, T=2048, E=1024, H=16, D=64) on 8 TRN2 cores.

Sharding: tensor-parallel over heads. Core c owns heads {2c, 2c+1} for both
batches. Each core computes its heads' q/k/v projections, causal attention,
and a partial output projection z_c = out_c @ Wo[:, 128c:128c+128].T.
Host combines: z = sum_c z_c + bo.

Note the reference computes wei = K @ Q^T, i.e. output token t attends over
s <= t with logits k_t . q_s. We compute ST[s, t] = q_s . k_t (s on
partitions) so that the A@V matmul needs no transposes, and get the softmax
denominator via a ones-column appended to V.
"""

import numpy as np
import ml_dtypes

import concourse.bacc as bacc
import concourse.mybir as mybir
import concourse.tile as tile
from concourse.bass_utils import run_bass_kernel_spmd
from concourse.masks import make_identity


def _make_runner(nc):
    """Persistent jitted SPMD callable (avoids per-call jit re-trace)."""
    import jax
    from jax.sharding import Mesh, NamedSharding, PartitionSpec
    try:
        from jax.experimental.shard_map import shard_map
    except ImportError:
        shard_map = jax.shard_map
    from concourse.bass2jax import (_bass_exec_p, install_neuronx_cc_hook,
                                    partition_id_tensor)

    install_neuronx_cc_hook()
    partition_name = (nc.partition_id_tensor.name
                      if nc.partition_id_tensor else None)
    in_names, out_names, out_avals, zero_outs = [], [], [], []
    for alloc in nc.m.functions[0].allocations:
        if not isinstance(alloc, mybir.MemoryLocationSet):
            continue
        name = alloc.memorylocations[0].name
        if alloc.kind == "ExternalInput":
            if name != partition_name:
                in_names.append(name)
        elif alloc.kind == "ExternalOutput":
            shape = tuple(alloc.tensor_shape)
            dtype = mybir.dt.np(alloc.dtype)
            out_names.append(name)
            out_avals.append(jax.core.ShapedArray(shape, dtype))
            zero_outs.append(np.zeros(shape, dtype))
    n_params = len(in_names)
    all_in = list(in_names) + list(out_names)
    if partition_name is not None:
        all_in.append(partition_name)

    def _body(*args):
        operands = list(args)
        if partition_name is not None:
            operands.append(partition_id_tensor())
        return tuple(_bass_exec_p.bind(
            *operands, out_avals=tuple(out_avals), in_names=tuple(all_in),
            out_names=tuple(out_names), lowering_input_output_aliases=(),
            sim_require_finite=True, sim_require_nnan=True, nc=nc))

    devices = jax.devices()[:N_CORES]
    mesh = Mesh(np.asarray(devices), ("core",))
    spec = NamedSharding(mesh, PartitionSpec("core"))
    rspec = NamedSharding(mesh, PartitionSpec())
    # inputs identical on every core are sent once and replicated
    replicated = {"xt", "mask"}
    in_specs = tuple(
        (PartitionSpec() if nm in replicated else PartitionSpec("core"))
        for nm in in_names) + (PartitionSpec("core"),) * len(out_names)
    fn = jax.jit(
        shard_map(_body, mesh=mesh, in_specs=in_specs,
                  out_specs=(PartitionSpec("core"),) * len(out_names),
                  check_rep=False),
        keep_unused=True)
    zeros_dev = [
        jax.device_put(np.zeros((N_CORES * z.shape[0], *z.shape[1:]), z.dtype),
                       spec) for z in zero_outs
    ]

    def run(in_maps):
        concat = [
            jax.device_put(np.asarray(in_maps[0][nm]), rspec)
            if nm in replicated else
            jax.device_put(
                np.concatenate([np.asarray(in_maps[c][nm])
                                for c in range(N_CORES)], axis=0), spec)
            for nm in in_names
        ]
        outs = fn(*concat, *zeros_dev)
        fulls = [np.asarray(outs[i]).reshape(N_CORES, *out_avals[i].shape)
                 for i in range(len(out_names))]
        return [{nm: fulls[i][c] for i, nm in enumerate(out_names)}
                for c in range(N_CORES)]

    return run

N_CORES = 8
B, T, E = 2, 2048, 1024
H, D = 16, 64
HPC = H // N_CORES          # heads per core = 2
F = HPC * D                 # local feature cols = 128
TBLK = 512                  # t-block width for stage A
NTB = T // TBLK             # 4
NSC = T // 128              # s-chunks = 16
NEC = E // 128              # e-chunks = 8
EXP_BIAS = -2.0             # exp(S + EXP_BIAS); cancels in softmax, guards overflow

F32 = mybir.dt.float32
F16 = mybir.dt.float16
F32R = mybir.dt.float32r
BF16 = mybir.dt.bfloat16
EXP = mybir.ActivationFunctionType.Exp


def build_nc(rep=1, cfg=None):
    cfg = dict(cfg or {})
    any_copy = cfg.get("any_copy", False)
    evict = cfg.get("evict", "mixed")  # mixed|zscalar|zvector
    sp_bufs = cfg.get("sp_bufs", 2)
    op_bufs = cfg.get("op_bufs", 2)
    misc_bufs = cfg.get("misc_bufs", None)  # if set, tp+zp merged [128,512] x misc_bufs
    pt_bufs = cfg.get("pt_bufs", 4)
    out_q = cfg.get("out_q", "scalar")  # engine for output DMAs
    xt_bf16 = cfg.get("xt_bf16", False)
    skip_z = cfg.get("skip_z", False)
    skip_b = cfg.get("skip_b", False)
    skip_attn = cfg.get("skip_attn", False)
    td_form = cfg.get("td_form", False)
    sp_wide = cfg.get("sp_wide", False)
    qk_bf16 = cfg.get("qk_bf16", False)   # bf16 qT2/kT2 + r=3 S trim
    v_bf16 = cfg.get("v_bf16", False)     # bf16 vT2 (faster PE transpose)
    exp_split = cfg.get("exp_split", False)  # exp only live cols of diag pair
    xtp_bufs = cfg.get("xtp_bufs", 36)
    xt_coal = cfg.get("xt_coal", False)  # one DMA per T-quarter (all E chunks)
    interleave = cfg.get("interleave", False)  # proj round 0 -> tb0/1 -> ...
    defer_z = cfg.get("defer_z", False)  # z-stage emitted inside next tb
    nc = bacc.Bacc("TRN2", target_bir_lowering=False, debug=False,
                   num_devices=N_CORES)

    xt = nc.dram_tensor("xt", [B, E, T], BF16 if xt_bf16 else F32R,
                        kind="ExternalInput").ap()
    # host pre-swizzles weights to [128, NEC*F]: w[p, e*F+f] = W[e*128+p, f]
    wdt = BF16 if xt_bf16 else F32R
    wq = nc.dram_tensor("wq", [128, NEC * F], wdt, kind="ExternalInput").ap()
    wk = nc.dram_tensor("wk", [128, NEC * F], wdt, kind="ExternalInput").ap()
    wv = nc.dram_tensor("wv", [128, NEC * F], wdt, kind="ExternalInput").ap()
    wot = nc.dram_tensor("wot", [F, E], F32R, kind="ExternalInput").ap()
    mask = nc.dram_tensor("mask", [128, 128], BF16, kind="ExternalInput").ap()
    zp = nc.dram_tensor("zp", [B, T, E], F16, kind="ExternalOutput").ap()

    with tile.TileContext(nc) as tc:
        with (
            tc.tile_pool(name="const", bufs=1) as cpool,
            tc.tile_pool(name="xtp", bufs=xtp_bufs) as xtp,
            tc.tile_pool(name="proj", bufs=2) as projp,
            tc.tile_pool(name="v2p", bufs=2 * NSC) as v2p,
            tc.tile_pool(name="ptp", bufs=pt_bufs) as ptp,
            tc.tile_pool(name="smallp", bufs=4) as smallp,
            tc.tile_pool(name="zsbp", bufs=3) as zsbp,
            tc.tile_pool(name="ps_s", bufs=sp_bufs, space="PSUM") as ps_s,
            tc.tile_pool(name="ps_o", bufs=op_bufs, space="PSUM") as ps_o,
            tc.tile_pool(name="ps_t", bufs=(misc_bufs or 2), space="PSUM") as ps_t,
        ):
            # ---- constants (loaded once) ----
            ident = cpool.tile([128, 128], F32, tag="ident")
            make_identity(nc, ident[:])
            if v_bf16:
                identb = cpool.tile([128, 128], BF16, tag="identb")
                make_identity(nc, identb[:])
            else:
                identb = ident
            ebias = cpool.tile([128, 1], F32, tag="ebias")
            nc.vector.memset(ebias[:], EXP_BIAS)
            # host pre-swizzled walls: straight [128, NEC*F] DMAs (2KB lines)
            wq_sb = []
            wk_sb = []
            wv_sb = []
            for lst, wsrc, nm in ((wq_sb, wq, "wq"), (wk_sb, wk, "wk"),
                                  (wv_sb, wv, "wv")):
                t_ = cpool.tile([128, NEC * F], BF16 if xt_bf16 else F32R,
                                tag=f"wall{nm}")
                nc.sync.dma_start(t_[:], wsrc)
                for e in range(NEC):
                    lst.append(t_[:, e * F:(e + 1) * F])
            wot_sb = cpool.tile([F, E], F32R, tag="wot")
            nc.sync.dma_start(wot_sb[:], wot)
            mask_sb = cpool.tile([128, 128], BF16, tag="mask")
            nc.sync.dma_start(mask_sb[:], mask)

            def body():
                pending = []

                def flush_pending():
                    while pending:
                        pending.pop(0)()

                xthb = {}
                if xt_coal:
                    # one coalesced DMA per T-quarter covering all 8 E-chunks
                    # (HWDGE setup is ~600ns serialized per dma_start); both
                    # batches issued up-front so batch 1's load overlaps
                    # batch 0's compute
                    for b in range(B):
                        xth = [[None] * 4 for _ in range(NEC)]
                        for qt in range(4):
                            t_ = xtp.tile([128, NEC * (T // 4)],
                                          BF16 if xt_bf16 else F32R,
                                          tag="xtq", bufs=8 if xt_bf16 else 5)
                            nc.sync.dma_start(
                                t_.rearrange("p (a c) -> p a c", a=NEC),
                                xt[b, :, qt * (T // 4):(qt + 1) * (T // 4)]
                                .rearrange("(a p) c -> p a c", p=128))
                            for e in range(NEC):
                                xth[e][qt] = t_[:, e * (T // 4):
                                                (e + 1) * (T // 4)]
                        xthb[b] = xth
                for b in range(B):
                    # ---- load transposed activations (t-halves so the first
                    # projection group can start after half the input DMA) ----
                    if xt_coal:
                        xth = xthb[b]
                    else:
                        xth = [[None] * 4 for _ in range(NEC)]
                        for qt in range(4):
                            for e in range(NEC):
                                t_ = xtp.tile([128, T // 4],
                                              BF16 if xt_bf16 else F32R,
                                              tag="xt")
                                nc.sync.dma_start(
                                    t_[:], xt[b, e * 128:(e + 1) * 128,
                                              qt * (T // 4):(qt + 1) * (T // 4)])
                                xth[e][qt] = t_

                    # ---- projections: qT2/kT2/vT2 [128(f), T] ----
                    heads = {}
                    for nm in ("q", "k", "v"):
                        if nm == "v":
                            pdt = BF16 if v_bf16 else F32
                        else:
                            pdt = BF16 if qk_bf16 else F32R
                        heads[nm] = projp.tile([128, T], pdt, tag=f"{nm}T2",
                                               name=f"{nm}T2")
                    qT2, kT2, vT2 = heads["q"], heads["k"], heads["v"]
                    v2 = [None] * NSC

                    def proj_round(tp2, flush_mid=False):
                        for gi, (nm, wsb) in enumerate(
                                (("q", wq_sb), ("k", wk_sb), ("v", wv_sb))):
                            if flush_mid and gi == 1:
                                flush_pending()
                            dst = heads[nm]
                            ps = ps_s.tile([128, 1024], F32, tag="sp",
                                           name="ps")
                            for half in range(2):
                                c0 = tp2 * 1024 + half * 512
                                for e in range(NEC):
                                    nc.tensor.matmul(
                                        ps[:, half * 512:(half + 1) * 512],
                                        wsb[e],
                                        xth[e][c0 // 512][:],
                                        start=(e == 0), stop=(e == NEC - 1))
                            (nc.any if any_copy else nc.vector).tensor_copy(
                                dst[:, tp2 * 1024:(tp2 + 1) * 1024], ps[:])

                    # ---- v2[s]: [128(s), 130] bf16 = [1|v_h0|1|v_h1] ----
                    def v2_round(s0, s1):
                        for s in range(s0, s1):
                            tpw = ps_t.tile([128, 512 if not v_bf16 else 1024],
                                            BF16 if v_bf16 else F32, tag="tp",
                                            name="tpw")
                            tp_ = tpw[:, 0:128]
                            nc.tensor.matmul(tp_[:],
                                             vT2[:, s * 128:(s + 1) * 128],
                                             identb[:], is_transpose=True)
                            v2t = v2p.tile([128, 130], BF16, tag="v2",
                                           name="v2t")
                            v2r = v2t.rearrange("p (g c) -> p g c", g=2)
                            nc.vector.memset(v2r[:, :, 64:65], 1.0)
                            nc.vector.tensor_copy(
                                v2r[:, :, 0:64],
                                tp_.rearrange("p (g c) -> p g c", g=2))
                            v2[s] = v2t

                    # ---- attention (stage B in outT form) ----
                    def attn_tb(tb):
                        slast = 4 * tb + 3
                        po = {}
                        for h in range(2 if not skip_b else 0):
                            if td_form:
                                po_t = ps_o.tile([128, 260], F32, tag="op")
                            else:
                                po_t = ps_o.tile([65, 512], F32, tag="op")
                            po[h] = po_t
                        npairs = 2 * tb + 2
                        for p in range(npairs):
                            if p == 1:
                                # previous t-block's deferred z-stage lands
                                # here so it doesn't head-of-line block this
                                # t-block's first S matmuls on the PE FIFO
                                flush_pending()
                            pts = []
                            for h in range(2):
                                ps = ps_s.tile([128, 1024], F32, tag="sp")
                                for dp in range(2):
                                    si = 2 * p + dp
                                    r = si - 4 * tb
                                    # trim causally-dead columns where fp32r
                                    # still streams at 1 cyc/row (N >= 256);
                                    # bf16 streams 1 cyc/row at any N
                                    trim_rs = (1, 2, 3) if qk_bf16 else (1, 2)
                                    c0 = 128 * r if r in trim_rs else 0
                                    nc.tensor.matmul(
                                        ps[:, dp * 512 + c0:(dp + 1) * 512],
                                        qT2[64 * h:64 * h + 64,
                                            si * 128:(si + 1) * 128],
                                        kT2[64 * h:64 * h + 64,
                                            tb * 512 + c0:(tb + 1) * 512],
                                        start=True, stop=True)
                                pt = ptp.tile([128, 1024], BF16, tag="pt")
                                if exp_split and p == 2 * tb + 1:
                                    # diagonal (r=2, r=3) pair: exp only the
                                    # causally-live columns of each chunk
                                    for dp in range(2):
                                        rr = 2 * p + dp - 4 * tb
                                        ec0 = dp * 512 + 128 * rr
                                        nc.scalar.activation(
                                            pt[:, ec0:(dp + 1) * 512],
                                            ps[:, ec0:(dp + 1) * 512], EXP,
                                            bias=ebias[:])
                                else:
                                    nc.scalar.activation(pt[:], ps[:], EXP,
                                                         bias=ebias[:])
                                for dp in range(2):
                                    si = 2 * p + dp
                                    r = si - 4 * tb
                                    if 0 <= r < 4:
                                        sl = pt[:, dp * 512 + r * 128:
                                                dp * 512 + (r + 1) * 128]
                                        meng = (nc.gpsimd if cfg.get("mask_pool")
                                                else nc.vector)
                                        meng.tensor_mul(sl, sl, mask_sb[:])
                                pts.append(pt)
                            for dp in range(2 if not skip_b else 0):
                                si = 2 * p + dp
                                for h in range(2):
                                    if td_form:
                                        # po[h] is [128(t), 260]: j-th chunk at
                                        # cols 130*(j%2); heads share tile pair
                                        for j in range(4):
                                            tcg = 4 * tb + j
                                            if si > tcg:
                                                continue
                                            jj = j % 2
                                            dst = po[j // 2][:, jj * 130 + h * 65:
                                                             jj * 130 + (h + 1) * 65]
                                            nc.tensor.matmul(
                                                dst,
                                                pts[h][:, dp * 512 + j * 128:
                                                       dp * 512 + (j + 1) * 128],
                                                v2[si][:, h * 65:(h + 1) * 65],
                                                start=(si == 0 and h == 0),
                                                stop=(si == tcg),
                                                skip_group_check=True)
                                    else:
                                        r = si - 4 * tb
                                        c0 = max(r, 0) * 128
                                        nc.tensor.matmul(
                                            po[h][:, c0:512],
                                            v2[si][:, h * 65:(h + 1) * 65],
                                            pts[h][:, dp * 512 + c0:
                                                   (dp + 1) * 512],
                                            start=(si == 0), stop=(si == slast),
                                            skip_group_check=True)

                        # ---- normalize (rows 1:65 / row 0) + partial z ----
                        if not skip_b and not skip_z:
                            outT = smallp.tile([128, 512], F32R, tag="outT")
                            for h in range(2):
                                rrow = smallp.tile([1, 512], F32, tag="rrow")
                                nc.vector.reciprocal(rrow[:], po[h][64:65, :])
                                rbc = smallp.tile([64, 512], F32, tag="rbc")
                                nc.gpsimd.partition_broadcast(rbc[:], rrow[:])
                                nc.vector.tensor_mul(
                                    outT[64 * h:64 * h + 64, :],
                                    po[h][0:64, :], rbc[:])

                            def z_stage(tb=tb, outT=outT, b=b):
                                z_stage_emit(tb, outT, b)

                            if defer_z:
                                pending.append(z_stage)
                            else:
                                z_stage()

                    def z_stage_emit(tb, outT, b):
                            for jp2 in range(2):
                                zsb = zsbp.tile([128, 2048], F16, tag="zsb")
                                for jj in range(2):
                                    j = 2 * jp2 + jj
                                    for eb in range(2):
                                        zps = ps_t.tile([128, 512], F32,
                                                        tag="tp")
                                        nc.tensor.matmul(
                                            zps[:],
                                            outT[:, j * 128:(j + 1) * 128],
                                            wot_sb[:, eb * 512:(eb + 1) * 512],
                                            start=True, stop=True)
                                        dstsl = zsb[:, jj * 1024 + eb * 512:
                                                    jj * 1024 + (eb + 1) * 512]
                                        if evict == "zscalar":
                                            nc.scalar.copy(dstsl, zps[:])
                                        elif evict == "zvector":
                                            nc.vector.tensor_copy(dstsl, zps[:])
                                        elif any_copy:
                                            nc.any.tensor_copy(dstsl, zps[:])
                                        elif eb == 0:
                                            nc.vector.tensor_copy(dstsl, zps[:])
                                        else:
                                            nc.scalar.copy(dstsl, zps[:])
                                t0r = (4 * tb + 2 * jp2) * 128
                                getattr(nc, out_q).dma_start(
                                    zp[b, t0r:t0r + 256, :]
                                    .rearrange("(a p) c -> p a c", p=128),
                                    zsb.rearrange("p (a c) -> p a c", a=2))

                    if interleave:
                        # attention for t-blocks 0/1 needs only the first
                        # projection round (cols 0-1023): emit it early so
                        # exp (ACT) overlaps the second projection round;
                        # end on tb=2 (6 pairs) instead of tb=3 (8) to
                        # shorten the drain tail
                        proj_round(0, flush_mid=True)
                        v2_round(0, 8)
                        if not skip_attn:
                            attn_tb(0)
                            attn_tb(1)
                        proj_round(1, flush_mid=True)
                        v2_round(8, NSC)
                        if not skip_attn:
                            attn_tb(3)
                            attn_tb(2)
                    else:
                        proj_round(0, flush_mid=True)
                        proj_round(1)
                        v2_round(0, NSC)
                        for tb in range(NTB if not skip_attn else 0):
                            attn_tb(tb)
                flush_pending()

            if rep == 1:
                body()
            else:
                with tc.For_i(0, rep, 1):
                    body()

    nc.compile()
    return nc


def make_in_maps(inputs, Wk, Wq, Wv, Wo, xt_bf16=False):
    """Shard full inputs into per-core input maps."""
    wdt = ml_dtypes.bfloat16 if xt_bf16 else np.float32
    xt = np.ascontiguousarray(inputs.transpose(0, 2, 1)).astype(wdt)
    scale = np.float32(D ** -0.5)
    tri = (np.arange(128)[None, :] >= np.arange(128)[:, None])
    mask = tri.astype(ml_dtypes.bfloat16)
    def swz(w):
        # [E, F] -> [128, NEC*F]: out[p, e*F+f] = w[e*128+p, f]
        nec = E // 128
        return np.ascontiguousarray(
            w.reshape(nec, 128, F).transpose(1, 0, 2).reshape(128, nec * F))

    in_maps = []
    for c in range(N_CORES):
        h0 = HPC * c
        wq2 = np.ascontiguousarray(
            np.concatenate([Wq[h0 + i] for i in range(HPC)], axis=1))
        wk2 = np.ascontiguousarray(
            np.concatenate([Wk[h0 + i] for i in range(HPC)], axis=1)) * scale
        wv2 = np.ascontiguousarray(
            np.concatenate([Wv[h0 + i] for i in range(HPC)], axis=1))
        wot = np.ascontiguousarray(Wo[:, F * c:F * (c + 1)].T)
        in_maps.append({
            "xt": xt,
            "wq": swz(wq2).astype(wdt),
            "wk": swz(wk2).astype(wdt),
            "wv": swz(wv2).astype(wdt),
            "wot": wot.astype(np.float32),
            "mask": mask,
        })
    return in_maps


_NC = None
_RUN = None
DEFAULT_CFG = {"any_copy": True, "out_q": "sync", "xt_bf16": False}


def kernel(inputs, Wk, Wq, Wv, Wo, bo):
    global _NC, _RUN
    if _NC is None:
        _NC = build_nc(cfg=DEFAULT_CFG)
    in_maps = make_in_maps(inputs, Wk, Wq, Wv, Wo,
                           xt_bf16=DEFAULT_CFG["xt_bf16"])
    try:
        if _RUN is None:
            _RUN = _make_runner(_NC)
        results = _RUN(in_maps)
    except Exception:
        _RUN = False if _RUN is None else _RUN
        res = run_bass_kernel_spmd(_NC, in_maps,
                                   core_ids=list(range(N_CORES)))
        results = res.results
    z = np.zeros((B, T, E), dtype=np.float32)
    for c in range(N_CORES):
        z += results[c]["zp"].astype(np.float32)
    return z + bo.astype(np.float32)

